# revision 19
# baseline (speedup 1.0000x reference)
"""CapsuleLayer (dynamic routing, N_IN=512, N_OUT=2, D=16, 3 iters) on 8 trn2
NeuronCores, pure data-parallel over the batch.

v4: host pre-transposes/casts u to [(ns,j)=128p, (g,b)] bf16; single block
BLK=512 per core; every matmul runs in full 128x128 PE mode (Z and W lhsT
zero-padded) so the PE never switches tiling mode; one shared PSUM quad tag
(bufs=3) cycles z-quads and dsum-quads; issue order is software-pipelined
(z(k) | dsum(k-1) | t(k-2)) so PE/ACT/DVE overlap; iter-2 logits use
z(v0+v1) (linearity) instead of re-streaming uz1.

softmax over k=2 == sigmoid of logit diff; squash(s) = g(|s|^2)*s.
"""

import numpy as np

N_CORES = 8
B = 4096
B_LOCAL = B // N_CORES          # 512
BLK = B_LOCAL                   # single block per core
N_IN, N_OUT, D = 512, 2, 16
G = 64                          # n-groups of 8 capsules: 8*16 = 128 partitions
NQ = G // 4                     # 16 quads of 4 groups
KI = N_OUT * D                  # 32
EPS = 1e-07
# of every 16 z-evac pairs, this many go to DVE (rest on ACT)
Z_EVAC_DVE = 9
# pairs with k % 8 in this set run their uz/uw multiplies on GPSIMD
GPSIMD_PAIRS = (3, 7)

_CACHE = {}


# ---------------------------------------------------------------------------
# Walrus in this container allows only ONE sync-wait per TPB instruction.
# Tile attaches several sem waits to one instruction; split extras onto
# standalone NoOps (same engine, one wait each) inserted just before it.
# ---------------------------------------------------------------------------
def _apply_tile_patch():
    import concourse.tile as tile_mod
    from concourse import mybir
    from concourse.vector_clock import ScopedClock
    from concourse._compat import nn

    if getattr(tile_mod.TileContext, "_wait_split_patched", False):
        return

    _orig_add_instruction = tile_mod.TileContext._add_instruction

    def _split_waits(self, inst):
        si = inst.sync_info
        if si is None or len(si.on_wait) <= 1:
            return
        waits = list(si.on_wait)
        ups = list(si.on_update)
        inst.sync_info = mybir.SyncInfo(on_wait=[waits[-1]], on_update=ups)
        for i, w in enumerate(waits[:-1]):
            nop = mybir.InstNoOp(name=f"{inst.name}-wsplit{i}", ins=[], outs=[])
            nop.engine = inst.engine
            nop.sync_info = mybir.SyncInfo(on_wait=[w], on_update=[])
            self.nc.register_instruction(nop, overwrite=True)
            nn(self.nc.cur_bb).bb.add_instruction(nop)

    def _patched_add_instruction(self, inst):
        _split_waits(self, inst)
        _orig_add_instruction(self, inst)

    def _patched_drain_and_barrier(self, tick_clock, wait_clock):
        nc = self.nc
        drain_inst = nc.sync.drain()
        wait_clock.add_sem_waits(
            drain_inst.ins, ScopedClock({None: tick_clock.global_clock})
        )
        si = drain_inst.ins.sync_info
        if si is not None and len(si.on_wait) > 1:
            waits = list(si.on_wait)
            ups = list(si.on_update)
            drain_inst.ins.sync_info = mybir.SyncInfo(
                on_wait=[waits[0]], on_update=ups
            )
            for w in waits[1:]:
                nop = nc.sync.nop(nofuse=True)
                nop.ins.sync_info = mybir.SyncInfo(on_wait=[w], on_update=[])

        nc.all_engine_barrier()
        assert self.sems is not None
        popped = nc._tile_sem_poison_stack.pop()
        assert popped is self._sem_poison
        nc.clear_and_free_semaphores(list(self.sems.allocated().values()))
        nc.all_engine_barrier()

    tile_mod.TileContext._add_instruction = _patched_add_instruction
    tile_mod.TileContext._drain_and_barrier = _patched_drain_and_barrier
    tile_mod.TileContext._wait_split_patched = True


# ---------------------------------------------------------------------------
# Host-side constant prep from W  (W: [1, 512, 2, 16, 16] f32, idx [_,n,k,i,j])
# ---------------------------------------------------------------------------
def _prep_consts(W):
    import ml_dtypes

    bf16 = ml_dtypes.bfloat16
    W = np.asarray(W, dtype=np.float32).reshape(N_IN, N_OUT, D, D)  # [n,k,i,j]
    Wg = W.reshape(G, 8, N_OUT, D, D)                   # [g, ns, k, i, j]
    # wpad[(ns,j), g, 0:32] = W[8g+ns, k, i, j]; cols 32:128 zero
    wpad = np.zeros((128, G, 128), dtype=np.float32)
    wpad[:, :, :KI] = np.transpose(Wg, (1, 4, 0, 2, 3)).reshape(128, G, KI)
    # zpad[0:32, g, (ns,j)] = sign(k) * W[8g+ns, k, i, j]; rows 32:128 zero
    zfull = np.transpose(Wg, (0, 2, 3, 1, 4)).reshape(G, KI, 128).copy()
    zfull[:, D:, :] *= -1.0                             # k=1 rows negative
    zpad = np.zeros((128, G, 128), dtype=np.float32)
    zpad[:KI] = np.transpose(zfull, (1, 0, 2))
    # OnesRep[(ns,j), (ns',j')] = 1 iff ns==ns'
    onesdiag = np.kron(np.eye(8, dtype=np.float32), np.ones((D, D), np.float32))
    # OnesK[(k,i), (k',i')] = 1 iff k==k'
    onesk = np.kron(np.eye(2, dtype=np.float32), np.ones((D, D), np.float32))
    km = np.concatenate([np.ones(D, np.float32), -np.ones(D, np.float32)])[:, None]
    pmsk = np.concatenate([np.zeros(D, np.float32), np.ones(D, np.float32)])[:, None]
    return {
        "wpad": np.ascontiguousarray(wpad.astype(bf16)),      # [128, 64, 128]
        "zpad": np.ascontiguousarray(zpad.astype(bf16)),      # [128, 64, 128]
        "onesdiag": np.ascontiguousarray(onesdiag.astype(bf16)),  # [128, 128]
        "onesk": np.ascontiguousarray(onesk.astype(bf16)),    # [32, 32]
        "kmask": np.ascontiguousarray(km),                    # [32, 1] f32
        "pmask": np.ascontiguousarray(pmsk),                  # [32, 1] f32
    }


def _prep_u(inputs):
    """Full inputs [B, 8,8,8,16] f32 -> per-core ut [128, G*BLK] bf16."""
    import ml_dtypes

    bf16 = ml_dtypes.bfloat16
    u = np.asarray(inputs, dtype=np.float32).reshape(B, N_IN * D)
    uts = []
    for c in range(N_CORES):
        slab = u[c * B_LOCAL : (c + 1) * B_LOCAL].astype(bf16)  # [512, 8192]
        ut = slab.reshape(B_LOCAL, G, 128).transpose(2, 1, 0)   # [p, g, b]
        uts.append(np.ascontiguousarray(ut.reshape(128, G * BLK)))
    return uts


def make_in_maps(inputs, W):
    consts = _prep_consts(W)
    uts = _prep_u(inputs)
    in_maps = []
    for c in range(N_CORES):
        m = {"ut": uts[c]}
        m.update(consts)
        in_maps.append(m)
    return in_maps


# ---------------------------------------------------------------------------
# Bass program
# ---------------------------------------------------------------------------
def _build_program(repeat=1):
    import contextlib

    import concourse.bass as bass
    import concourse.tile as tile
    from concourse import mybir

    _apply_tile_patch()
    f32 = mybir.dt.float32
    bf16 = mybir.dt.bfloat16

    nc = bass.Bass(trn_type="TRN2", target_bir_lowering=False)
    ut_in = nc.declare_dram_parameter("ut", [128, G * BLK], bf16, isOutput=False)
    wpad_in = nc.declare_dram_parameter("wpad", [128, G, 128], bf16, isOutput=False)
    zpad_in = nc.declare_dram_parameter("zpad", [128, G, 128], bf16, isOutput=False)
    onesdiag_in = nc.declare_dram_parameter(
        "onesdiag", [128, 128], bf16, isOutput=False
    )
    onesk_in = nc.declare_dram_parameter("onesk", [KI, KI], bf16, isOutput=False)
    kmask_in = nc.declare_dram_parameter("kmask", [KI, 1], f32, isOutput=False)
    pmask_in = nc.declare_dram_parameter("pmask", [KI, 1], f32, isOutput=False)
    v_out = nc.declare_dram_parameter("v", [KI, B_LOCAL], f32, isOutput=True)

    Sig = mybir.ActivationFunctionType.Sigmoid
    Sqrt = mybir.ActivationFunctionType.Sqrt

    with tile.TileContext(nc) as tc:
        with contextlib.ExitStack() as ctx:
            consts = ctx.enter_context(tc.tile_pool(name="consts", bufs=1))
            ut_p = ctx.enter_context(tc.tile_pool(name="ut", bufs=1))
            za_p = ctx.enter_context(tc.tile_pool(name="za", bufs=4))
            uz_p = ctx.enter_context(tc.tile_pool(name="uz", bufs=4))
            sg_p = ctx.enter_context(tc.tile_pool(name="sg", bufs=4))
            uw_p = ctx.enter_context(tc.tile_pool(name="uw", bufs=4))
            sm_p = ctx.enter_context(tc.tile_pool(name="sm", bufs=1))
            vp_p = ctx.enter_context(tc.tile_pool(name="vp", bufs=2))
            ps_p = ctx.enter_context(tc.tile_pool(name="ps", bufs=1, space="PSUM"))

            # --- constants to SBUF (outside repeat loop)
            wpad = consts.tile([128, G, 128], bf16)
            nc.sync.dma_start(out=wpad, in_=wpad_in[:, :, :])
            zpad = consts.tile([128, G, 128], bf16)
            nc.sync.dma_start(out=zpad, in_=zpad_in[:, :, :])
            onesdiag = consts.tile([128, 128], bf16)
            nc.sync.dma_start(out=onesdiag, in_=onesdiag_in[:, :])
            onesk = consts.tile([KI, KI], bf16)
            nc.sync.dma_start(out=onesk, in_=onesk_in[:, :])
            kmask = consts.tile([KI, 1], f32)
            nc.sync.dma_start(out=kmask, in_=kmask_in[:, :])
            pmask = consts.tile([KI, 1], f32)
            nc.sync.dma_start(out=pmask, in_=pmask_in[:, :])

            rep_cm = tc.For_i(0, repeat, 1) if repeat > 1 else contextlib.nullcontext()

            def squash(s_sb, tag):
                """s_sb [KI, BLK] f32 -> vt f32 [KI, BLK]."""
                s2 = sm_p.tile([KI, BLK], bf16, tag="s2", name=f"s2{tag}")
                nc.scalar.square(s2, s_sb)
                nsq = ps_p.tile([KI, BLK], f32, tag="sm", name=f"nsq{tag}")
                nc.tensor.matmul(nsq, onesk, s2, start=True, stop=True)
                sqr = sm_p.tile([KI, BLK], f32, tag="sqr", name=f"sqr{tag}")
                nc.scalar.activation(sqr, nsq, Sqrt)
                nc.vector.tensor_scalar_add(sqr, sqr, EPS)
                den = sm_p.tile([KI, BLK], f32, tag="den", name=f"den{tag}")
                # den = (nsq + 1) * sqr
                nc.vector.scalar_tensor_tensor(
                    out=den, in0=nsq, scalar=1.0, in1=sqr,
                    op0=mybir.AluOpType.add, op1=mybir.AluOpType.mult,
                )
                nc.vector.reciprocal(den, den)
                gfac = sm_p.tile([KI, BLK], f32, tag="gfac", name=f"gfac{tag}")
                nc.vector.tensor_mul(gfac, nsq, den)
                vt = vp_p.tile([KI, BLK], f32, tag="vt", name=f"vt{tag}")
                nc.vector.tensor_mul(vt, s_sb, gfac)
                return vt

            def replicate_v(vt, tag):
                """vt [KI, BLK] f32 -> vrep [128, BLK] bf16 (4 row copies)."""
                vrep = vp_p.tile([128, BLK], bf16, tag="vrep", name=f"vrep{tag}")
                for r in range(4):
                    nc.vector.tensor_copy(out=vrep[32 * r : 32 * r + 32, :], in_=vt)
                return vrep

            with rep_cm:
                # --- load ut in 4 chunks (overlap with P-phase)
                ut = ut_p.tile([128, G, BLK], bf16, tag="ut")
                for q4 in range(4):
                    nc.sync.dma_start(
                        out=ut[:, 16 * q4 : 16 * (q4 + 1), :],
                        in_=ut_in[:, 16 * BLK * q4 : 16 * BLK * (q4 + 1)],
                    )

                # --- P-phase: P^T[ki, b] = sum_g Wpad[g]^T @ uT[g] (rows 0:32)
                pacc = ps_p.tile([128, BLK], f32, tag="acc", name="paccP")
                for g in range(G):
                    nc.tensor.matmul(
                        pacc, wpad[:, g, :], ut[:, g, :],
                        start=(g == 0), stop=(g == G - 1),
                    )
                p_sb = sm_p.tile([KI, BLK], f32, tag="psb")
                nc.vector.tensor_copy(out=p_sb, in_=pacc[:KI, :])
                pmp = sm_p.tile([KI, BLK], f32, tag="pmp")
                nc.vector.tensor_scalar_mul(pmp, p_sb, pmask)
                s1 = sm_p.tile([KI, BLK], f32, tag="s1")
                nc.vector.tensor_scalar_mul(s1, p_sb, 0.5)
                vt_prev = squash(s1, "i0")       # v0 (f32)
                vsum0 = None

                for it in range(2):
                    if it == 0:
                        vrep = replicate_v(vt_prev, "i0")
                        vsum0 = vt_prev
                    else:
                        # logits are linear in v: use v0 + v1 for iter 2
                        vsum = sm_p.tile([KI, BLK], f32, tag="vsum")
                        nc.vector.tensor_add(vsum, vsum0, vt_prev)
                        vrep = replicate_v(vsum, "i1")

                    tacc = ps_p.tile([128, BLK], f32, tag="acc", name=f"tacc{it}")

                    # software-pipelined over 32 pairs of 2 groups:
                    # step k issues z(k), dsum(k-1), t(k-2)
                    NP = G // 2
                    za = {}
                    uz = {}
                    sig = {}
                    uw = {}
                    for k in range(NP + 2):
                        if k < NP:
                            # z-pair k: z[g] = Zpad_g^T @ vrep  (rows 0:32 live)
                            zq = ps_p.tile(
                                [128, 2, BLK], f32, tag="q", bufs=3,
                                name=f"zq{it}_{k}",
                            )
                            for c in range(2):
                                nc.tensor.matmul(
                                    zq[:, c, :], zpad[:, 2 * k + c, :], vrep,
                                    start=True, stop=True,
                                )
                            zs = za_p.tile(
                                [128, 2, BLK], bf16, tag="zac", name=f"za{it}_{k}"
                            )
                            if k % 16 < Z_EVAC_DVE:
                                nc.vector.tensor_copy(out=zs, in_=zq)
                            else:
                                nc.scalar.copy(out=zs, in_=zq)
                            za[k] = zs
                            gs = slice(2 * k, 2 * k + 2)
                            uzk = uz_p.tile(
                                [128, 2, BLK], bf16, tag="uzc", name=f"uz{it}_{k}"
                            )
                            if k % 8 in GPSIMD_PAIRS:
                                nc.gpsimd.tensor_mul(uzk, ut[:, gs, :], zs)
                            else:
                                nc.vector.tensor_mul(uzk, ut[:, gs, :], zs)
                            uz[k] = uzk
                        if 1 <= k <= NP:
                            kk = k - 1
                            dq = ps_p.tile(
                                [128, 2, BLK], f32, tag="q", bufs=3,
                                name=f"dq{it}_{kk}",
                            )
                            for c in range(2):
                                nc.tensor.matmul(
                                    dq[:, c, :], onesdiag, uz[kk][:, c, :],
                                    start=True, stop=True,
                                )
                            sgk = sg_p.tile(
                                [128, 2, BLK], bf16, tag="sgc", name=f"sg{it}_{kk}"
                            )
                            nc.scalar.activation(sgk, dq, Sig)
                            sig[kk] = sgk
                            gs = slice(2 * kk, 2 * kk + 2)
                            uwk = uw_p.tile(
                                [128, 2, BLK], bf16, tag="uwc", name=f"uw{it}_{kk}"
                            )
                            if kk % 8 in GPSIMD_PAIRS:
                                nc.gpsimd.tensor_mul(uwk, ut[:, gs, :], sgk)
                            else:
                                nc.vector.tensor_mul(uwk, ut[:, gs, :], sgk)
                            uw[kk] = uwk
                        if k >= 2:
                            kk = k - 2
                            for c in range(2):
                                g = 2 * kk + c
                                nc.tensor.matmul(
                                    tacc, wpad[:, g, :], uw[kk][:, c, :],
                                    start=(g == 0), stop=(g == G - 1),
                                )

                    # s = t*kmask + P*pmask
                    s_sb = sm_p.tile([KI, BLK], f32, tag="ssb", name=f"ssb{it}")
                    nc.vector.scalar_tensor_tensor(
                        out=s_sb, in0=tacc[:KI, :], scalar=kmask, in1=pmp,
                        op0=mybir.AluOpType.mult, op1=mybir.AluOpType.add,
                    )
                    vt_prev = squash(s_sb, f"i{it + 1}")

                nc.sync.dma_start(out=v_out[:, :], in_=vt_prev)

    return nc


def _get_program(repeat=1, cast_via_dma=True):
    key = ("nc", repeat)
    if key not in _CACHE:
        _CACHE[key] = _build_program(repeat)
    return _CACHE[key]


# ---------------------------------------------------------------------------
# Public entry: full inputs -> full output
# ---------------------------------------------------------------------------
def kernel(inputs, W):
    from concourse.bass_utils import run_bass_kernel_spmd

    in_maps = make_in_maps(inputs, W)
    nc = _get_program()
    res = run_bass_kernel_spmd(nc, in_maps, list(range(N_CORES)))
    outs = []
    for c in range(N_CORES):
        vt = res.results[c]["v"]                  # [KI, B_LOCAL]
        outs.append(vt.T)                         # [B_LOCAL, KI]
    v = np.concatenate(outs, axis=0)              # [B, 32]
    return np.ascontiguousarray(v.reshape(B, 1, N_OUT, D).astype(np.float32))
